# revision 41
# baseline (speedup 1.0000x reference)
"""BiLSTM-CRF negative log-likelihood on 8 Trainium2 NeuronCores.

Strategy (two launches + host marshaling):
  L1: each LSTM direction is split into 4*LANES time chunks; 4 cores per
      direction advance LANES chunks in lockstep as extra rhs columns of the
      weight-stationary recurrence matvecs (weight streaming dominates, so
      lanes are nearly free). Each chunk warm-starts WARM steps early from a
      cold state (LSTM state contracts, validated ~2e-3 end-to-end).
      Recurrent matmuls use fp8 DoubleRow (2 contraction k-tiles per
      instruction); xp input projections are computed once per UNIQUE input
      position (lanes overlap) and stay SBUF-resident; the per-step xp term
      enters PSUM via a bf16 identity matmul; h state/output is fp8.
      Embedding gather happens on the host (input marshaling).
  L2: 8 cores shard the 4096 timesteps; per core the CRF partition chunk
      runs as 8 sub-chains of 64 steps in two block-diagonal groups of 4
      (4x32 tags = 128 partitions): per iter one [128,128] bf16 matmul with
      the CONSTANT exp(trans + lin_b') matrix + a scaled PSUM->SBUF copy on
      the scalar engine (activation Copy with per-partition scale = exp(em)).
      No renormalization (the -ln 32 shift keeps factors O(1)). Emissions are
      computed straight into band layout via zero-masked banded lin_w tiles.
      NOTE this stack's measured pathologies, avoided here: activations with
      a bias AP (~18us each), DVE tensor_scalar with an AP scalar in a chain
      (~21us), tile_position'd matmuls (~11us), SBUF->SBUF cross-partition
      DMA (~100us).
  L3: the 32x32 band-matrix chain, logZ and the CRF score run on the HOST
      in float64 (trivial numpy work, saves a launch).
"""

import numpy as np
import ml_dtypes

import bass_rust
import jax
from jax.experimental.shard_map import shard_map
from jax.sharding import Mesh, PartitionSpec

import concourse.bass as bass
import concourse.bass_isa as bass_isa
import concourse.mybir as mybir
import concourse.tile as tile
from concourse.vector_clock import ScopedClock
from concourse import bass2jax
from concourse.bass2jax import install_neuronx_cc_hook, _bass_exec_p
from concourse.masks import make_identity

# ---------------------------------------------------------------------------
# Workaround: this walrus build rejects >1 sem-wait on CTRL-class (Drain)
# instructions. Split the TileContext tail-drain's waits onto dedicated
# single-wait nops.
# ---------------------------------------------------------------------------


def _patched_drain_and_barrier(self, tick_clock, wait_clock):
    nc = self.nc
    dummy = nc.sync.nop(nofuse=True, hint="tail_wait_collector")
    wait_clock.add_sem_waits(dummy.ins, ScopedClock({None: tick_clock.global_clock}))
    si = dummy.ins.sync_info
    if si is not None and len(si.on_wait) > 1:
        waits = list(si.on_wait)
        dummy.ins.sync_info = bass_rust.SyncInfo(
            on_wait=waits[:1], on_update=list(si.on_update)
        )
        for w in waits[1:]:
            n = nc.sync.nop(nofuse=True, hint="tail_wait_split")
            n.ins.sync_info = bass_rust.SyncInfo(on_wait=[w], on_update=[])
    nc.sync.drain()
    nc.all_engine_barrier()
    assert self.sems is not None
    popped = nc._tile_sem_poison_stack.pop()
    assert popped is self._sem_poison
    nc.clear_and_free_semaphores(list(self.sems.allocated().values()))
    nc.all_engine_barrier()


tile.TileContext._drain_and_barrier = _patched_drain_and_barrier


def _split_multi_waits(nc):
    """This walrus build allows only one sync-wait per instruction. Hoist
    extra waits onto same-engine single-wait nops placed just before."""
    ctr = 0
    for f in nc.m.functions:
        for bb in f.blocks:
            insts = bb.instructions
            if not any(
                i.sync_info is not None and len(i.sync_info.on_wait) > 1
                for i in insts
            ):
                continue
            out = []
            for inst in insts:
                si = inst.sync_info
                if si is not None and len(si.on_wait) > 1:
                    waits = list(si.on_wait)
                    for w in waits[:-1]:
                        n = mybir.InstNoOp(name=f"waitsplit_{ctr}", ins=[], outs=[])
                        ctr += 1
                        n.engine = inst.engine
                        n.sync_info = bass_rust.SyncInfo(on_wait=[w], on_update=[])
                        out.append(n)
                    inst.sync_info = bass_rust.SyncInfo(
                        on_wait=[waits[-1]], on_update=list(si.on_update)
                    )
                out.append(inst)
            bb.instructions = out
    return nc

# ---------------------------------------------------------------------------
# Problem constants
# ---------------------------------------------------------------------------
V, E, HID, T, S = 50000, 512, 1024, 32, 4096
H = HID // 2          # 512 per-direction hidden
P = 128
NCORES = 8
G4 = 4 * H            # 2048 gate rows
NMC = G4 // P         # 16 gate chunks
NK = H // P           # 4 hidden chunks
LN32 = float(np.log(32.0))

F32 = mybir.dt.float32
BF16 = mybir.dt.bfloat16
I32 = mybir.dt.int32
AF = mybir.ActivationFunctionType
BF16NP = ml_dtypes.bfloat16

# recurrent-weight dtype: fp8e4m3 + DoubleRow perf mode (2 k-tiles per
# matmul instruction — halves PE instruction count in the recurrence)
FP8 = mybir.dt.float8e4
FP8NP = ml_dtypes.float8_e4m3
DR = mybir.MatmulPerfMode.DoubleRow

# Time-parallel L1: each direction is split into 4*LANES chunks; each of the
# 4 cores per direction advances LANES chunks in lockstep (extra rhs columns
# in the recurrence matvecs, nearly free since weight load dominates). Each
# chunk re-runs WARM extra leading steps from a cold state; the LSTM state
# contracts fast (forget gates ~0.5), so warm-started states converge to the
# exact trajectory well within WARM steps (validated against the reference).
LANES = 64
WARM = 32


def _l1_dims(lanes=None, warm=None):
    lanes = LANES if lanes is None else lanes
    warm = WARM if warm is None else warm
    nch = 4 * lanes             # chunks per direction
    chunk = S // nch
    run = warm + chunk          # steps per core
    tot = run * lanes           # gather/xp positions per core
    bl = min(tot, 512)          # xp columns per recurrence block
    b = bl // lanes             # steps per recurrence block
    if b > 128:
        b, bl = 128, 128 * lanes
    assert run % b == 0 and tot % P == 0 and tot % min(tot, 512) == 0
    return nch, chunk, run, tot, b, bl


def _gate_perm():
    """Row permutation taking PyTorch gate order [i f g o] x H to our
    M-chunk order: mc = half*8 + c with per-half cols [i0 i1 f0 f1 o0 o1 g0 g1]
    (hc = half*2 + (c&1), sigmoid cols 0:6, tanh cols 6:8)."""
    qmap = [0, 0, 1, 1, 3, 3, 2, 2]  # i i f f o o g g  (PyTorch q: i=0 f=1 g=2 o=3)
    order = []
    for half in (0, 1):
        for c in range(8):
            q = qmap[c]
            hc = half * 2 + (c & 1)
            base = q * H + hc * P
            order.append(np.arange(base, base + P))
    return np.concatenate(order)


# ---------------------------------------------------------------------------
# Persistent-executable runner (adapted from bass2jax.run_bass_via_pjrt)
# ---------------------------------------------------------------------------
class Prog:
    def __init__(self, nc: bass.Bass, n_cores: int = NCORES):
        install_neuronx_cc_hook()
        self.nc = nc
        self.n_cores = n_cores
        in_names, out_names, out_avals, zero_outs = [], [], [], []
        partition_name = (
            nc.partition_id_tensor.name if nc.partition_id_tensor else None
        )
        for alloc in nc.m.functions[0].allocations:
            if not isinstance(alloc, mybir.MemoryLocationSet):
                continue
            name = alloc.memorylocations[0].name
            if alloc.kind == "ExternalInput":
                if name != partition_name:
                    in_names.append(name)
            elif alloc.kind == "ExternalOutput":
                out_names.append(name)
                shape = tuple(alloc.tensor_shape)
                dtype = mybir.dt.np(alloc.dtype)
                out_avals.append(jax.core.ShapedArray(shape, dtype))
                zero_outs.append(np.zeros(shape, dtype))
        assert nc.dbg_addr is None
        self.in_names, self.out_names = in_names, out_names
        self.out_avals, self.zero_outs = out_avals, zero_outs
        n_params, n_outs = len(in_names), len(out_names)
        all_names = in_names + out_names
        if partition_name is not None:
            all_names = all_names + [partition_name]
        donate = tuple(range(n_params, n_params + n_outs))

        def _body(*args):
            operands = list(args)
            if partition_name is not None:
                operands.append(bass2jax.partition_id_tensor())
            return tuple(
                _bass_exec_p.bind(
                    *operands,
                    out_avals=tuple(out_avals),
                    in_names=tuple(all_names),
                    out_names=tuple(out_names),
                    lowering_input_output_aliases=(),
                    sim_require_finite=False,
                    sim_require_nnan=False,
                    nc=nc,
                )
            )

        devices = jax.devices()[:n_cores]
        self.mesh = Mesh(np.asarray(devices), ("core",))
        in_specs = (PartitionSpec("core"),) * (n_params + n_outs)
        out_specs = (PartitionSpec("core"),) * n_outs
        self.sharded = jax.jit(
            shard_map(
                _body,
                mesh=self.mesh,
                in_specs=in_specs,
                out_specs=out_specs,
                check_rep=False,
            ),
            donate_argnums=donate,
            keep_unused=True,
        )
        self._dev_in = None

    def stage(self, in_maps):
        """device_put the concatenated per-core inputs once."""
        from jax.sharding import NamedSharding

        sh = NamedSharding(self.mesh, PartitionSpec("core"))
        concat = [
            np.concatenate([np.asarray(in_maps[c][n]) for c in range(self.n_cores)], 0)
            for n in self.in_names
        ]
        self._dev_in = [jax.device_put(a, sh) for a in concat]

    def _zeros_dev(self):
        from jax.sharding import NamedSharding

        sh = NamedSharding(self.mesh, PartitionSpec("core"))
        return [
            jax.device_put(
                np.zeros((self.n_cores * z.shape[0], *z.shape[1:]), z.dtype), sh
            )
            for z in self.zero_outs
        ]

    def run(self):
        assert self._dev_in is not None
        zs = self._zeros_dev()
        outs = self.sharded(*self._dev_in, *zs)
        outs = [np.asarray(o) for o in outs]
        return [
            {
                n: outs[i].reshape(self.n_cores, *self.out_avals[i].shape)[c]
                for i, n in enumerate(self.out_names)
            }
            for c in range(self.n_cores)
        ]

    def time_exec(self, iters=3):
        """Median wall time of a warm execution (device-resident inputs)."""
        import time

        ts = []
        for _ in range(iters):
            zs = self._zeros_dev()
            for z in zs:
                z.block_until_ready()
            t0 = time.perf_counter()
            outs = self.sharded(*self._dev_in, *zs)
            for o in outs:
                o.block_until_ready()
            ts.append(time.perf_counter() - t0)
        return float(np.median(ts))


# ---------------------------------------------------------------------------
# L1: embedding gather + input projection + one LSTM direction per core
# ---------------------------------------------------------------------------
def build_l1(run, lanes, V_=V):
    """One LSTM direction, `lanes` chunks advancing in lockstep per core.

    Lane l's step s reads input position A + l*C + s of this core's UNIQUE
    id window (A = core_base - warm; host clips negative ids to 0 — the
    affected warm-from-garbage outputs decay away within a few steps).
    Consecutive lanes overlap by `warm` positions, so the embedding gather
    and input projections are computed once per unique position (UC of them)
    and the recurrence reads lane slices with stride C. xp stays SBUF-
    resident. The recurrence is fully unrolled (no hardware loop).
    """
    LW = lanes
    C = S // (4 * lanes)  # chunk (useful steps per lane)
    TOT = run * LW        # hout cols (col index = s*LW + l)
    UC = lanes * C + (run - C)        # unique positions per core
    UCP = -(-UC // P) * P             # padded to gather-block multiple
    B = 512 // LW         # steps per hist/hout block
    BL = B * LW           # = 512 cols per block
    NB = run // B         # recurrence blocks
    nc = bass.Bass("TRN2", target_bir_lowering=False, debug=False, num_devices=NCORES)
    # embeddings pre-gathered AND pre-transposed on the host (input
    # marshaling): xTin[e, p, u] = emb[ids[u], e*128 + p]
    xt_ap = nc.dram_tensor("xTin", [E // P, P, UCP], BF16,
                           kind="ExternalInput").ap()
    wihT_ap = nc.dram_tensor("wihT", [E, G4], BF16, kind="ExternalInput").ap()
    # whh as DoubleRow pairs: [pair, p, t, gates], h-dim = (2*pair + t)*128 + p
    whhT_ap = nc.dram_tensor("whhT", [2, P, 2, G4], FP8, kind="ExternalInput").ap()
    b_ap = nc.dram_tensor("b", [P, NMC], F32, kind="ExternalInput").ap()
    hout_ap = nc.dram_tensor("houtT", [NK, P, TOT], FP8, kind="ExternalOutput").ap()

    with tile.TileContext(nc) as tc:
        with tc.tile_pool(name="const", bufs=1) as constp, \
             tc.tile_pool(name="ps", bufs=2, space="PSUM") as psp, \
             tc.tile_pool(name="gsb", bufs=3) as gsbp, \
             tc.tile_pool(name="tmp2", bufs=4) as tmpp:

            ident = constp.tile([P, P], BF16, tag="ident")
            make_identity(nc, ident[:])

            # resident weights
            wih_e = []
            for e in range(E // P):
                t_ = constp.tile([P, G4], BF16, tag=f"wih{e}")
                nc.sync.dma_start(t_[:], wihT_ap[bass.ts(e, P), :])
                wih_e.append(t_)
            whh_pr = []
            for pr_ in range(2):
                t_ = constp.tile([P, 2, G4], FP8, tag=f"whh{pr_}")
                nc.sync.dma_start(t_[:], whhT_ap[pr_, :, :, :])
                whh_pr.append(t_)
            b_sb = constp.tile([P, NMC], F32, tag="bias")
            nc.sync.dma_start(b_sb[:], b_ap[:])

            # ---- x planes (pre-gathered/transposed on host): [128e, UCP] ----
            xT = []
            for e in range(E // P):
                t_ = constp.tile([P, UCP], BF16, tag=f"xT{e}")
                nc.sync.dma_start(t_[:], xt_ap[e, :, :])
                xT.append(t_)

            # ---- input projections -> SBUF-resident xp [P, mc, u] bf16 ----
            xp_sb = constp.tile([P, NMC, UCP], BF16, tag="xp")
            for off in range(0, UCP, 512):
                w_ = min(512, UCP - off)
                for mc in range(NMC):
                    ps = psp.tile([P, 512], F32, tag="xpps")
                    for e in range(E // P):
                        nc.tensor.matmul(
                            ps[:, 0:w_],
                            lhsT=wih_e[e][:, bass.ts(mc, P)],
                            rhs=xT[e][:, off : off + w_],
                            start=(e == 0),
                            stop=(e == E // P - 1),
                        )
                    nc.vector.tensor_scalar_add(
                        xp_sb[:, mc, off : off + w_], ps[:, 0:w_],
                        b_sb[:, mc : mc + 1],
                    )

            # ---- recurrence state (LW lanes side by side) ----
            # hist[half][:, chd, s*LW + l] holds h chunk (2*half + chd), lane l
            # at step s, in fp8 — it is both the recurrence state (DoubleRow
            # matmul rhs) and the block output (DMA to hout).
            cbuf = []
            for half in (0, 1):
                t_ = constp.tile([P, 2 * LW], F32, tag=f"c{half}")
                cbuf.append(t_)
            hist = []
            for half in (0, 1):
                t_ = constp.tile([P, 2, BL], FP8, tag=f"hist{half}")
                hist.append(t_)
            nc.vector.memset(hist[0][:, :, BL - LW : BL], 0.0)
            nc.vector.memset(hist[1][:, :, BL - LW : BL], 0.0)
            nc.vector.memset(cbuf[0][:], 0.0)
            nc.vector.memset(cbuf[1][:], 0.0)

            for s in range(run):
                sb_ = s % B
                sl = slice(sb_ * LW, (sb_ + 1) * LW)
                pv = slice((sb_ - 1) % B * LW, ((sb_ - 1) % B + 1) * LW)
                # per half: one PSUM tile accumulates xp (identity matmul,
                # bf16) + both DoubleRow whh contraction pairs (fp8)
                psG = [None, None]
                for half in (0, 1):
                    psG[half] = psp.tile([P, 8 * LW], F32, tag=f"g{half}",
                                         name=f"g{half}_t")
                    nc.tensor.matmul(
                        psG[half][:],
                        lhsT=ident[:],
                        rhs=xp_sb[:, half * 8 : (half + 1) * 8,
                                  s : s + (LW - 1) * C + 1 : C],
                        start=True, stop=False, skip_group_check=True,
                    )
                    for c in range(8):
                        nc.tensor.matmul(
                            psG[half][:, c * LW : (c + 1) * LW],
                            lhsT=whh_pr[0][:, :, bass.ts(half * 8 + c, P)],
                            rhs=hist[0][:, :, pv],
                            start=False, stop=False, perf_mode=DR,
                            skip_group_check=True,
                        )
                for half in (0, 1):
                    for c in range(8):
                        nc.tensor.matmul(
                            psG[half][:, c * LW : (c + 1) * LW],
                            lhsT=whh_pr[1][:, :, bass.ts(half * 8 + c, P)],
                            rhs=hist[1][:, :, pv],
                            start=False, stop=True, perf_mode=DR,
                            skip_group_check=True,
                        )
                for half in (0, 1):
                    sg = gsbp.tile([P, 8 * LW], F32, tag="gact")
                    nc.scalar.activation(sg[:, 0 : 6 * LW],
                                         psG[half][:, 0 : 6 * LW], AF.Sigmoid)
                    nc.scalar.activation(sg[:, 6 * LW : 8 * LW],
                                         psG[half][:, 6 * LW : 8 * LW], AF.Tanh)
                    ig = tmpp.tile([P, 2 * LW], F32, tag="ig")
                    nc.vector.tensor_mul(ig[:], sg[:, 0 : 2 * LW],
                                         sg[:, 6 * LW : 8 * LW])
                    nc.vector.tensor_mul(cbuf[half][:], sg[:, 2 * LW : 4 * LW],
                                         cbuf[half][:])
                    nc.vector.tensor_add(cbuf[half][:], cbuf[half][:], ig[:])
                    th = tmpp.tile([P, 2 * LW], F32, tag="th")
                    nc.scalar.activation(th[:], cbuf[half][:], AF.Tanh)
                    nc.vector.tensor_mul(
                        hist[half][:, :, sl], sg[:, 4 * LW : 6 * LW], th[:]
                    )
                if (s + 1) % B == 0:
                    blk = (s + 1) // B - 1
                    for half in (0, 1):
                        for chd in (0, 1):
                            nc.sync.dma_start(
                                hout_ap[2 * half + chd, :,
                                        blk * BL : (blk + 1) * BL],
                                hist[half][:, chd, :],
                            )
    return _split_multi_waits(nc)


# ---------------------------------------------------------------------------
# L2: emissions + CRF chunk products + score partials (t sharded 8 ways)
# ---------------------------------------------------------------------------
def build_l2(S_=S):
    """Emissions + CRF chunk products, 8 sub-chains per core.

    The core's SC=512 steps split into 8 sub-chains of SUB=64 steps. Two
    block-diagonal groups of 4 sub-chains (each 4x32=128 partitions) run
    interleaved: per iter, each group applies one scaled-exp transition
    matrix [128,128] (bf16) to its stacked state RT_g [128,32]. No
    renormalization needed: the -ln(32) shift in lb keeps factors O(1), so
    64-step products stay comfortably in range. The 8 resulting 32x32 band
    matrices are combined on the HOST (tiny numpy chain), as is the rest of
    the CRF loss — no L3 launch.
    """
    SC = S_ // NCORES     # timesteps per core
    NH = HID // P         # 8 hid chunks
    NG = 2                # block-diag groups
    SUB = SC // (NG * 4)  # steps per sub-chain
    nc = bass.Bass("TRN2", target_bir_lowering=False, debug=False, num_devices=NCORES)
    hT_ap = nc.dram_tensor("hT", [NH, P, SC], FP8, kind="ExternalInput").ap()
    # banded lin_w: lwTb[a, k*128+p, 32a+j] = lin_w.T[k*128+p, j], 0 elsewhere.
    # A matmul with this lhsT writes band a's rows and adds 0 to the rest, so
    # emissions land directly in block-diag band layout with tile_position
    # (0,0) (explicitly-positioned matmuls measured ~11us each on this stack).
    lwb_ap = nc.dram_tensor("lwTb", [4, HID, P], FP8, kind="ExternalInput").ap()
    # Ebd = exp(block-diag(trans) + lb' per row), f32, precomputed on host
    ebd_ap = nc.dram_tensor("Ebd", [P, P], F32, kind="ExternalInput").ap()
    ohtbd_ap = nc.dram_tensor("ohtbd", [P, NG * SUB], F32,
                              kind="ExternalInput").ap()
    foldid_ap = nc.dram_tensor("foldid", [P, T], F32, kind="ExternalInput").ap()
    # packed output: cols [0:32]=RT_0 bands, [32:64]=RT_1 bands,
    # [64:96]=R7p (sub-7 product excl. last factor), [96]=raw-score partials,
    # [97]=raw em last cols — host adds the lb' parts back
    l2out_ap = nc.dram_tensor("l2out", [P, 98], F32, kind="ExternalOutput").ap()

    with tile.TileContext(nc) as tc:
        with tc.tile_pool(name="const", bufs=1) as constp, \
             tc.tile_pool(name="emps", bufs=2, space="PSUM") as empsp, \
             tc.tile_pool(name="crfps", bufs=2, space="PSUM") as crfpsp, \
             tc.tile_pool(name="misc", bufs=2) as miscp:

            lwb = []
            for a in range(4):
                row = []
                for k in range(NH):
                    t_ = constp.tile([P, P], FP8, tag=f"lwb{a}_{k}")
                    nc.sync.dma_start(t_[:], lwb_ap[a, bass.ts(k, P), :])
                    row.append(t_)
                lwb.append(row)
            h_k = []
            for k in range(NH):
                t_ = constp.tile([P, SC], FP8, tag=f"h{k}")
                nc.sync.dma_start(t_[:], hT_ap[k, :, :])
                h_k.append(t_)
            ebd_sb = constp.tile([P, P], F32, tag="ebd")
            nc.sync.dma_start(ebd_sb[:], ebd_ap[:])
            ohtbd_sb = constp.tile([P, NG * SUB], F32, tag="ohtbd")
            nc.sync.dma_start(ohtbd_sb[:], ohtbd_ap[:])
            foldid_sb = constp.tile([P, T], F32, tag="foldid")
            nc.sync.dma_start(foldid_sb[:], foldid_ap[:])
            ebf = constp.tile([P, P], BF16, tag="ebf")
            nc.vector.tensor_copy(ebf[:], ebd_sb[:])

            # raw emissions straight into band layout: for group g,
            # emps_g[32a+j, s] = em_raw[(4g+a)*SUB + s, j]
            out_all = constp.tile([P, 98], F32, tag="outall")
            EEM, emr = [], []
            for g in range(NG):
                emps = empsp.tile([P, SUB], F32, tag="emps")
                n_mm = 0
                for a in range(4):
                    u = g * 4 + a
                    for k in range(NH):
                        nc.tensor.matmul(
                            emps[:],
                            lhsT=lwb[a][k][:],
                            rhs=h_k[k][:, u * SUB : (u + 1) * SUB],
                            start=(n_mm == 0), stop=(n_mm == 4 * NH - 1),
                        )
                        n_mm += 1
                eemg = constp.tile([P, SUB], F32, tag=f"eem{g}")
                nc.scalar.activation(eemg[:], emps[:], AF.Exp)
                EEM.append(eemg)
                emrg = constp.tile([P, SUB], F32, tag=f"emr{g}")
                nc.vector.tensor_copy(emrg[:], emps[:])
                emr.append(emrg)

            # raw score partials: rs[p] = sum_s em_raw[p, s] * onehot[p, s]
            rparts = []
            for g in range(NG):
                prod = miscp.tile([P, SUB], F32, tag="prod")
                nc.vector.tensor_mul(prod[:], emr[g][:],
                                     ohtbd_sb[:, g * SUB : (g + 1) * SUB])
                r_ = miscp.tile([P, 1], F32, tag=f"r{g}")
                nc.vector.tensor_reduce(
                    r_[:], prod[:], axis=mybir.AxisListType.X,
                    op=mybir.AluOpType.add,
                )
                rparts.append(r_)
            nc.vector.tensor_add(out_all[:, 96:97], rparts[0][:], rparts[1][:])
            nc.vector.tensor_copy(out_all[:, 97:98], emr[1][:, SUB - 1 : SUB])

            # interleaved block-diag chains; Q_s = diag(eem_{s+1}) @ RT_s so
            # each iter is: pr = Ebd^T @ Q (PE) + scaled copy on the scalar
            # engine (activation Copy with per-partition scale — the only
            # fast per-row broadcast multiply on this stack).
            Q = []
            for g in range(NG):
                t_ = constp.tile([P, T], BF16, tag=f"Q{g}")
                nc.scalar.activation(t_[:], foldid_sb[:], AF.Copy,
                                     scale=EEM[g][:, 0:1])
                Q.append(t_)
            for s_ in range(SUB):
                for g in range(NG):
                    pr = crfpsp.tile([P, T], F32, tag=f"pr{g}")
                    nc.tensor.matmul(pr[:], lhsT=ebf[:], rhs=Q[g][:],
                                     start=True, stop=True)
                    if s_ < SUB - 1:
                        nc.scalar.activation(Q[g][:], pr[:], AF.Copy,
                                             scale=EEM[g][:, s_ + 1 : s_ + 2])
                        if s_ == SUB - 2 and g == NG - 1:
                            nc.vector.tensor_copy(out_all[:, 2 * T : 3 * T],
                                                  pr[:])
                    else:
                        nc.vector.tensor_copy(
                            out_all[:, g * T : (g + 1) * T], pr[:]
                        )
            nc.sync.dma_start(l2out_ap[:], out_all[:])
    return _split_multi_waits(nc)


# ---------------------------------------------------------------------------
# Host orchestration
# ---------------------------------------------------------------------------
_progs = {}


def _get_prog(key, builder):
    if key not in _progs:
        _progs[key] = Prog(builder())
    return _progs[key]


def _wpack(wih, whh, b):
    perm = _gate_perm()
    wihT = np.ascontiguousarray(wih[perm].T).astype(BF16NP)   # [E, 2048]
    whhT = np.ascontiguousarray(whh[perm].T)                  # [H, 2048]
    # DoubleRow pair layout [pair, p, t, gates]; h-dim = (2*pair + t)*128 + p
    whh_dr = np.ascontiguousarray(
        whhT.reshape(2, 2, P, G4).transpose(0, 2, 1, 3)
    ).astype(FP8NP)
    b_re = np.ascontiguousarray(b[perm].reshape(NMC, P).T).astype(np.float32)
    return wihT, whh_dr, b_re


def _prep_l1_maps_chunked(input_ids, emb, wf, whf, bf, wb, whb, bb):
    """Full-scale path: cores 0-3 forward lanes, 4-7 backward lanes.
    Each core gets the UNIQUE id window [core_base - warm, core_base + L*C),
    clipped into range (core 0's leading warm ids degenerate to ids[0];
    the garbage-warm error decays away within a few steps)."""
    _, chunk, run, _, _, _ = _l1_dims()
    warm = run - chunk
    uc = LANES * chunk + warm
    ucp = -(-uc // P) * P
    ids32 = np.asarray(input_ids).astype(np.int64).reshape(S)
    ids_rev = ids32[::-1].copy()
    emb_bf = np.asarray(emb).astype(BF16NP)
    wihT_f, whhT_f, b_f_re = _wpack(np.asarray(wf), np.asarray(whf), np.asarray(bf))
    wihT_b, whhT_b, b_b_re = _wpack(np.asarray(wb), np.asarray(whb), np.asarray(bb))
    maps = []
    for d, (idsd, wi, wh, bb_) in enumerate(
        ((ids32, wihT_f, whhT_f, b_f_re), (ids_rev, wihT_b, whhT_b, b_b_re))
    ):
        for j in range(4):
            a = j * LANES * chunk - warm
            idx = np.clip(a + np.arange(ucp), 0, S - 1)
            xg = emb_bf[idsd[idx]]                       # [ucp, E]
            xt = np.ascontiguousarray(xg.T).reshape(E // P, P, ucp)
            maps.append({
                "xTin": xt,
                "wihT": wi,
                "whhT": wh,
                "b": bb_,
            })
    return maps


def _stitch_chunks(r1):
    _, chunk, run, _, _, _ = _l1_dims()
    warm = run - chunk

    def stitch(rows):
        parts = []
        for core in rows:
            lanes = core.reshape(NK, P, run, LANES)
            for l in range(LANES):
                parts.append(lanes[:, :, warm : warm + chunk, l])
        return np.concatenate(parts, axis=2)

    hfT = stitch([r1[c]["houtT"] for c in range(4)])
    hbT = stitch([r1[4 + c]["houtT"] for c in range(4)])[:, :, ::-1]
    return hfT, hbT


def _prep_l2_maps(hfT, hbT, lin_w, lin_b, target, trans_np, S_=S):
    SC = S_ // NCORES
    NG, SUB = 2, SC // 8
    h_allT = np.concatenate([hfT, hbT], axis=0)  # [8, 128, S_] fp8
    lwT = np.ascontiguousarray(np.asarray(lin_w).T)          # [HID, T] f32
    lwb = np.zeros((4, HID, P), np.float32)
    for a in range(4):
        lwb[a, :, 32 * a : 32 * a + 32] = lwT
    lwb = lwb.astype(FP8NP)
    lb = np.asarray(lin_b).astype(np.float64) - LN32
    ebd = np.zeros((P, P), np.float32)
    blk = np.exp(np.asarray(trans_np, np.float64) + lb[:, None]).astype(np.float32)
    for a in range(4):
        ebd[32 * a : 32 * a + 32, 32 * a : 32 * a + 32] = blk
    foldid = np.tile(np.eye(T, dtype=np.float32), (4, 1))
    maps = []
    for c in range(NCORES):
        tgt = np.asarray(target[c * SC : (c + 1) * SC]).astype(np.int64)
        tgt = tgt.reshape(8, SUB)
        oht = np.zeros((P, NG * SUB), np.float32)
        for u in range(8):
            g, a = u // 4, u % 4
            oht[32 * a + tgt[u], g * SUB + np.arange(SUB)] = 1.0
        maps.append({
            "hT": np.ascontiguousarray(h_allT[:, :, c * SC : (c + 1) * SC]),
            "lwTb": lwb,
            "Ebd": ebd,
            "ohtbd": oht,
            "foldid": foldid,
        })
    return maps


def kernel(input_ids, target, emb, wih_f, whh_f, b_f, wih_b, whh_b, b_b,
           lin_w, lin_b, start_trans, end_trans, trans, _S=S, _V=V):
    input_ids = np.asarray(input_ids)
    target = np.asarray(target).astype(np.int64)
    trans_np = np.asarray(trans).astype(np.float32)

    # ---- L1: two LSTM directions ----
    assert _S == S and _V == V
    _, _, run, _, _, _ = _l1_dims()
    p1 = _get_prog(("l1", run, LANES, _V), lambda: build_l1(run, LANES, _V))
    p1.stage(_prep_l1_maps_chunked(input_ids, emb, wih_f, whh_f, b_f,
                                   wih_b, whh_b, b_b))
    r1 = p1.run()
    hfT, hbT = _stitch_chunks(r1)

    # ---- L2: emissions + CRF chunks ----
    p2 = _get_prog(("l2", _S), lambda: build_l2(_S))
    maps2 = _prep_l2_maps(hfT, hbT, lin_w, lin_b, target, trans_np, S_=_S)
    p2.stage(maps2)
    r2 = p2.run()

    # ---- host combine: chain the 64 band matrices, CRF score, loss ----
    l2o = [r2[c]["l2out"].astype(np.float64) for c in range(NCORES)]
    logacc = 0.0
    v = np.exp(np.asarray(start_trans, np.float64))
    for c in range(NCORES):
        o = l2o[c]
        bands = [o[32 * a : 32 * a + 32, g * T : (g + 1) * T]
                 for g in range(2) for a in range(4)]
        if c == NCORES - 1:
            bands[7] = o[96:128, 2 * T : 3 * T]
        for u in range(8):
            v = bands[u] @ v
            m = v.max()
            v /= m
            logacc += np.log(m)
    # device emissions/scores are raw (no bias): add the lb' = lin_b - ln(32)
    # parts back on the host (they were folded into Ebd for the chains).
    lbp = np.asarray(lin_b, np.float64) - LN32
    emlast = l2o[-1][96:128, 97] + lbp
    ev = np.asarray(end_trans, np.float64)
    logZ = np.log((v * np.exp(emlast + ev)).sum()) + logacc
    score_em = sum(float(o[:, 96].sum()) for o in l2o) + float(lbp[target].sum())
    score = (float(np.asarray(start_trans)[target[0]]) + score_em
             + float(np.asarray(trans_np)[target[:-1], target[1:]].sum())
             + float(np.asarray(end_trans)[target[-1]]))
    return np.float32(logZ - score).reshape(())

